# revision 1
# baseline (speedup 1.0000x reference)
"""CRF layer (fc -> CRF log-likelihood + Viterbi decode) on 8 NeuronCores.

Data-parallel over batch: B=256 is sharded 32-per-core across the 8
cores; fc weights and the tiny CRF transition tables are replicated.
The per-core program (matmul + forward scan + Viterbi scan + backtrace)
is compiled once via jax.pmap onto the neuron devices.

kernel(**inputs) takes FULL unsharded inputs and returns the FULL
output: (-log_likelihood (f32 scalar), tags [256,256] int32).
"""
import numpy as np
import jax
import jax.numpy as jnp

B, S, D, T = 256, 256, 768, 21
M = 8          # cores
BL = B // M    # 32 batch rows per core


def _per_shard(features, labels, crf_masks, fc_w, fc_b,
               start_transitions, end_transitions, transitions):
    # features [BL,S,D] f32; labels [BL,S] i32; crf_masks [BL,S] bool
    emissions = features @ fc_w + fc_b  # [BL,S,T]
    maskf = crf_masks.astype(emissions.dtype)

    # ----- numerator: score of gold path -----
    tags0 = labels[:, 0]
    score = start_transitions[tags0] + \
        jnp.take_along_axis(emissions[:, 0], tags0[:, None], 1)[:, 0]
    prev, cur = labels[:, :-1], labels[:, 1:]
    trans_sc = transitions[prev, cur]  # [BL,S-1]
    em_sc = jnp.take_along_axis(emissions[:, 1:], cur[..., None], 2)[..., 0]
    score = score + jnp.sum((trans_sc + em_sc) * maskf[:, 1:], axis=1)
    seq_ends = crf_masks.sum(1) - 1
    last_tags = jnp.take_along_axis(labels, seq_ends[:, None], 1)[:, 0]
    numerator = score + end_transitions[last_tags]

    # ----- denominator: log partition via scan -----
    alpha0 = start_transitions[None, :] + emissions[:, 0]  # [BL,T]
    em_rest = jnp.swapaxes(emissions[:, 1:], 0, 1)  # [S-1,BL,T]
    m_rest = jnp.swapaxes(crf_masks[:, 1:], 0, 1)   # [S-1,BL]

    def step(alpha, inp):
        em, m = inp
        nxt = jax.nn.logsumexp(
            alpha[:, :, None] + transitions[None] + em[:, None, :], axis=1)
        return jnp.where(m[:, None], nxt, alpha), None

    alpha, _ = jax.lax.scan(step, alpha0, (em_rest, m_rest))
    denominator = jax.nn.logsumexp(alpha + end_transitions[None, :], axis=1)
    llh_part = jnp.sum(numerator - denominator)

    # ----- Viterbi decode -----
    idty = jnp.arange(T)[None, :]

    def vstep(score_c, inp):
        em, m = inp
        brd = score_c[:, :, None] + transitions[None]  # [BL,T,T]
        nxt = jnp.max(brd, axis=1) + em
        idx = jnp.argmax(brd, axis=1)  # [BL,T]
        new_score = jnp.where(m[:, None], nxt, score_c)
        idx = jnp.where(m[:, None], idx, idty)
        return new_score, idx

    vscore, history = jax.lax.scan(vstep, alpha0, (em_rest, m_rest))
    last_tag = jnp.argmax(vscore + end_transitions[None, :], axis=1)  # [BL]

    def bstep(tag, hist):
        tag = jnp.take_along_axis(hist, tag[:, None], 1)[:, 0]
        return tag, tag

    _, rev_tags = jax.lax.scan(bstep, last_tag, history, reverse=True)
    tags = jnp.concatenate([rev_tags, last_tag[None, :]], axis=0)  # [S,BL]
    return llh_part, jnp.swapaxes(tags, 0, 1).astype(jnp.int32)


_pmapped = jax.pmap(
    _per_shard,
    in_axes=(0, 0, 0, None, None, None, None, None),
)


def kernel(features, fc_w, fc_b, labels, crf_masks,
           start_transitions, end_transitions, transitions):
    features = np.asarray(features, dtype=np.float32).reshape(M, BL, S, D)
    labels_s = np.asarray(labels, dtype=np.int32).reshape(M, BL, S)
    masks_s = np.asarray(crf_masks, dtype=bool).reshape(M, BL, S)
    fc_w = np.asarray(fc_w, dtype=np.float32)
    fc_b = np.asarray(fc_b, dtype=np.float32)
    start_transitions = np.asarray(start_transitions, dtype=np.float32)
    end_transitions = np.asarray(end_transitions, dtype=np.float32)
    transitions = np.asarray(transitions, dtype=np.float32)

    llh_parts, tags = _pmapped(
        features, labels_s, masks_s, fc_w, fc_b,
        start_transitions, end_transitions, transitions)
    neg_llh = np.float32(-np.sum(np.asarray(llh_parts), dtype=np.float64))
    tags_full = np.asarray(tags).reshape(B, S).astype(np.int32)
    return neg_llh, tags_full
